# revision 38
# baseline (speedup 1.0000x reference)
"""Multi-head attention (B=4, S=2048, D=512, H=8) on 8 trn2 NeuronCores.

Sharding: core c handles batch b=c//2, head-group g=c%2 (4 heads, 256 of the
512 projection dims). Each core runs the full fused pipeline for its four
heads — QKV projection, scores^T = K_h Q_h^T, exp (softmax numerator),
attn @ V with a folded ones-column producing the softmax denominators,
normalization, and its partial output projection y^T = Wo_slice^T.T @ O^T.
The host sums the two partial y^T per batch and adds the output bias.

All attention matmuls run in bf16 with fp32 PSUM accumulation; scores^T is
computed transposed (keys on partitions) so the exp'd tiles feed the V
contraction directly with no on-chip transposes. exp skips max-subtraction:
scaled scores are ~N(0,1) (|x| < ~7 over this problem's distribution), far
inside fp32 exp range, and bf16 numerator storage is sum-normalized later.

The exp stream on the scalar engine (~16.8M elements/core) is the critical
resource; every pool coexists for the whole kernel (PSUM banks: 2 proj +
4 score + 2 attn@V = 8) so the score->exp chain starts as soon as the Q/K
projections for its head pair land and never waits on a pool boundary.
"""

import re

import numpy as np
import ml_dtypes

import concourse.bass as bass
import concourse.mybir as mybir
from concourse.bass_utils import run_bass_kernel_spmd
from concourse.tile import ScopedClock, TileContext, VectorClock

BF16 = mybir.dt.bfloat16
F32 = mybir.dt.float32
F32R = mybir.dt.float32r
NP_BF16 = ml_dtypes.bfloat16

B, S, D, H, DK = 4, 2048, 512, 8, 64
SCALE = float(1.0 / (np.float32(np.sqrt(DK)) + 1e-8))
# Schraudolph bf16-bits exp on the DVE: int16(x*SA + SB) reinterpreted as
# bf16 is exp(x*SCALE) to within a +-4.7% sawtooth. A couple of key-tiles
# per head ride this (one tensor_scalar op) to take load off the scalar
# engine's exp stream; the softmax denominator (ones-column) is computed
# from the same approximated values, so the error stays a bounded
# reweighting of 1/8 of the attention mass (~0.8% rms on the output).
SCHRA_A = float(128.0 * np.log2(np.e) * SCALE)
SCHRA_B = 16253.596
# Measured on HW: offloading kt tiles (5, 13) saved only ~1-2us (the exp
# stream is not purely ACT-gated at those points) while raising the output
# error from 9.3e-3 to 1.7e-2 — nearly the whole 2e-2 budget. Disabled.
DVE_KT = ()
E = 256          # head dims per core (4 heads)
NCORES = 8
KT = S // 128    # 16 key tiles of 128
QB = 2           # q blocks of 1024
SB = S // 512    # 4 s-blocks of 512

Exp = mybir.ActivationFunctionType.Exp
Ln = mybir.ActivationFunctionType.Ln


# ---------------------------------------------------------------------------
# walrus in this container rejects >1 sync-wait command per instruction;
# split the Tile tail drain and hoist excess mid-kernel waits onto NoOps.
# ---------------------------------------------------------------------------

def _clock_entries(vc):
    nums = [int(s) for s in re.findall(r"-?\d+", repr(vc))]
    return [(i, n) for i, n in enumerate(nums) if n > 0]


class SplitDrainTileContext(TileContext):
    def _drain_and_barrier(self, tick_clock, wait_clock):
        nc = self.nc
        for proc, tick in _clock_entries(tick_clock.global_clock):
            vc = VectorClock()
            vc.require_at_least(proc, tick)
            carrier = nc.sync.nop()
            wait_clock.add_sem_waits(carrier.ins, ScopedClock({None: vc}))
        nc.sync.drain()
        nc.all_engine_barrier()
        assert self.sems is not None
        popped = nc._tile_sem_poison_stack.pop()
        assert popped is self._sem_poison
        nc.clear_and_free_semaphores(list(self.sems.allocated().values()))
        nc.all_engine_barrier()


def sanitize_waits(nc, max_waits: int = 1):
    n_split = 0
    for fn in nc.m.functions:
        for bb in fn.blocks:
            new_insts = []
            for inst in bb.instructions:
                si = inst.sync_info
                waits = list(si.on_wait) if si and si.on_wait else []
                if len(waits) > max_waits:
                    keep = waits[-max_waits:]
                    excess = waits[:-max_waits]
                    for i in range(0, len(excess), max_waits):
                        nop = mybir.InstNoOp(
                            name=nc.get_next_instruction_name(), ins=[], outs=[]
                        )
                        nop.engine = inst.engine
                        nop.sync_info = mybir.SyncInfo(
                            on_wait=excess[i : i + max_waits], on_update=[]
                        )
                        new_insts.append(nop)
                    inst.sync_info = mybir.SyncInfo(
                        on_wait=keep, on_update=si.on_update
                    )
                    n_split += 1
                new_insts.append(inst)
            bb.instructions[:] = new_insts
    return n_split


# ---------------------------------------------------------------------------
# kernel builder (one SPMD program; per-core data differs only in in_maps)
# ---------------------------------------------------------------------------

def build_nc(sanitize=True, fast_recip=False, sel_k4=True):
    nc = bass.Bass("TRN2", target_bir_lowering=False, debug=False,
                   num_devices=NCORES)

    # x^T tensors arrive host-permuted as [128, 4, S]: partition p holds
    # d-rows {p, 128+p, 256+p, 384+p} so one DMA moves 16KB contiguous per
    # partition (4KB-row descriptors run at ~90GB/s/queue; 16KB near line
    # rate).
    xqT = nc.declare_dram_parameter("xqT", [128, 4, S], BF16, isOutput=False)
    xkT = nc.declare_dram_parameter("xkT", [128, 4, S], BF16, isOutput=False)
    xvT = nc.declare_dram_parameter("xvT", [128, 4, S], BF16, isOutput=False)
    wqT = nc.declare_dram_parameter("wqT", [D, E], BF16, isOutput=False)
    wkT = nc.declare_dram_parameter("wkT", [D, E], BF16, isOutput=False)
    wvT = nc.declare_dram_parameter("wvT", [D, E], BF16, isOutput=False)
    woT = nc.declare_dram_parameter("woT", [E, D], BF16, isOutput=False)
    bqs = nc.declare_dram_parameter("bqs", [E], F32, isOutput=False)
    bks = nc.declare_dram_parameter("bks", [E], F32, isOutput=False)
    bvb = nc.declare_dram_parameter("bvb", [128, E], F32, isOutput=False)
    e8d = nc.declare_dram_parameter("e8d", [8, 512], F32, isOutput=False)
    yT = nc.declare_dram_parameter("yT", [D, S], F32, isOutput=True)

    with SplitDrainTileContext(nc) as tc:
        with tc.sbuf_pool(name="persist", bufs=1) as P, \
             tc.sbuf_pool(name="xin", bufs=1) as X, \
             tc.sbuf_pool(name="ptp", bufs=34) as PTP, \
             tc.sbuf_pool(name="nrm", bufs=1) as NRM, \
             tc.sbuf_pool(name="yo", bufs=4) as YO, \
             tc.psum_pool(name="prj", bufs=2) as PRJ, \
             tc.psum_pool(name="scp", bufs=2) as SCP, \
             tc.psum_pool(name="opp", bufs=2) as OPP:
            QT = P.tile([128, 2, S], BF16)    # e-tiles x queries
            KTt = P.tile([128, 2, S], BF16)
            VA = P.tile([128, KT, 4 * 65], BF16)  # [V_h | ones] per head
            OT = P.tile([128, 2, S], BF16)
            WOT = P.tile([128, 2, D], BF16)
            BQ = P.tile([128, 2], F32)
            BK = P.tile([128, 2], F32)
            BVB = P.tile([128, E], F32)
            # E8[k, j*64+m] = (k==j): selector for broadcasting row j of an
            # [8, 512] tile across 64 partitions via a K=8 matmul.
            E8 = P.tile([8, 512], F32R)
            E8F = P.tile([8, 512], F32)
            # zeroed operand for tail warmth matmuls (results never read)
            WRM = P.tile([128, 256], BF16)

            XQT = X.tile([128, 4, S], BF16)
            XKT = X.tile([128, 4, S], BF16)
            XVT = X.tile([128, 4, S], BF16)
            WQ = X.tile([128, 4, E], BF16)
            WK = X.tile([128, 4, E], BF16)
            WVs = X.tile([128, 4, E], BF16)

            # ---- input DMAs ------------------------------------------------
            # K and Q feed the score->exp chain; give them the two HWDGE
            # rings exclusively at the start (ring order is FIFO), with the
            # V input queued behind them. Small constants ride SWDGE.
            for dt in range(4):
                sl = slice(dt * 128, (dt + 1) * 128)
                nc.sync.dma_start(out=WK[:, dt, :], in_=wkT[sl, :])
                nc.scalar.dma_start(out=WQ[:, dt, :], in_=wqT[sl, :])
            # two s-quarters then the back half: the first score tile needs
            # only K keys 0-511 and Q queries 0-1023, so finer completion
            # granularity on the front of the stream starts the exp chain
            # ~5us earlier than half-tensor loads
            for ssl2 in (slice(0, 512), slice(512, 1024), slice(1024, 2048)):
                nc.sync.dma_start(out=XKT[:, :, ssl2], in_=xkT[:, :, ssl2])
                nc.scalar.dma_start(out=XQT[:, :, ssl2], in_=xqT[:, :, ssl2])
            for half in range(2):
                hsl2 = slice(half * 2, half * 2 + 2)
                eng = nc.sync if half == 0 else nc.scalar
                eng.dma_start(out=XVT[:, hsl2, :], in_=xvT[:, hsl2, :])
            nc.gpsimd.dma_start(out=BVB[:, :], in_=bvb[:, :])
            nc.gpsimd.dma_start(
                out=BQ[:, :], in_=bqs[:].rearrange("(c p) -> p c", p=128)
            )
            nc.gpsimd.dma_start(
                out=BK[:, :], in_=bks[:].rearrange("(c p) -> p c", p=128)
            )
            nc.gpsimd.dma_start(out=E8F[:, :], in_=e8d[:, :])
            for dt in range(4):
                sl = slice(dt * 128, (dt + 1) * 128)
                nc.gpsimd.dma_start(out=WVs[:, dt, :], in_=wvT[sl, :])
            for et in range(2):
                sl = slice(et * 128, (et + 1) * 128)
                nc.gpsimd.dma_start(out=WOT[:, et, :], in_=woT[sl, :])

            with nc.allow_low_precision(reason="exact 0/1 rounded to fp32r"):
                nc.vector.tensor_copy(E8[:, :], E8F[:, :])
            nc.vector.memset(WRM[:, :], 0.0)
            # softmax-denominator ones columns of V_aug
            for kt in range(KT):
                va_h = VA[:, kt, :].rearrange("p (h c) -> p h c", c=65)
                nc.vector.memset(va_h[:, :, 64:65], 1.0)

            # ---- phase 1: QKV projections (PRJ psum, coexists with rest) ---
            # Q^T, K^T: [e, s] (head dims on partitions); e-tile 0 first so
            # head-pair (0,1) scores can begin before e-tile 1 projects, and
            # s-half 0 first so the psum-slot round-robin never queues a
            # second-half group (waiting on DMA) ahead of a ready one.
            for et in range(2):
                for sb in range(SB):
                    for xt, wt, out, bias in (
                        (XKT, WK, KTt, BK),
                        (XQT, WQ, QT, BQ),
                    ):
                        ssl = slice(sb * 512, (sb + 1) * 512)
                        ps = PRJ.tile([128, 512], F32, tag="prj")
                        for dt in range(4):
                            nc.tensor.matmul(
                                ps[:, :],
                                lhsT=wt[:, dt, et * 128:(et + 1) * 128],
                                rhs=xt[:, dt, ssl],
                                start=(dt == 0),
                                stop=(dt == 3),
                            )
                        nc.vector.tensor_scalar_add(
                            out[:, et, ssl], ps[:, :], bias[:, et:et + 1]
                        )

            # V: natural [s, e] + bias, interleaved [V_h | ones]
            for kt in range(KT):
                psv = PRJ.tile([128, 512], F32, tag="prj")
                for dt in range(4):
                    nc.tensor.matmul(
                        psv[:, 0:E],
                        lhsT=XVT[:, dt, kt * 128:(kt + 1) * 128],
                        rhs=WVs[:, dt, :],
                        start=(dt == 0),
                        stop=(dt == 3),
                    )
                va_v = VA[:, kt, :].rearrange("p (h c) -> p h c", c=65)
                bvb_v = BVB.rearrange("p (h c) -> p h c", c=64)
                nc.vector.tensor_add(
                    va_v[:, :, 0:64], psv[:, 0:E].rearrange(
                        "p (h c) -> p h c", c=64),
                    bvb_v[:, :, :],
                )

            # ---- attention + output projection, per 1024-query block -------
            # 1024-wide blocks keep the score/attn@V matmul cadence dense
            # enough that the PE's HAM clock-gate stays at full rate (512-
            # blocks measurably re-throttle it) and ACTIVATEs at N=1024.
            QBLOCKS = [(0, 1024), (1024, 1024)]
            NB = len(QBLOCKS)
            for bi, (q0, nq) in enumerate(QBLOCKS):
                nhf = nq // 512
                nj = 4 * nhf
                last = bi == NB - 1
                # unnormalized O tiles (row 64 = softmax denominator) and
                # the denominator rows of this query block. The final block
                # splits them per query-slice so each slice's reciprocal can
                # fire as soon as its own four rows land.
                if last:
                    suA = NRM.tile([8, 512], F32, tag="sums", bufs=2)
                    suB = NRM.tile([8, 512], F32, tag="sums", bufs=2)
                else:
                    sums = NRM.tile([8, 512], F32, tag="sums", bufs=2)
                ous = {}
                for hp in range(2):       # head pair = e-tile
                    et = hp
                    pts = {}
                    # scores + exp are the kernel's critical chain: let the
                    # scheduler prefer them over trailing attention matmuls
                    # (sc-slot backpressure still bounds run-ahead).
                    with tc.high_priority(offset=300):
                        for kt in range(KT):
                            scs = []
                            for hh in range(2):   # head within pair
                                hsl = slice(hh * 64, hh * 64 + 64)
                                sc = SCP.tile([128, 1024], F32, tag="sc")
                                for hf in range(nhf):
                                    nc.tensor.matmul(
                                        sc[:, hf * 512:(hf + 1) * 512],
                                        lhsT=KTt[hsl, et,
                                                 kt * 128:(kt + 1) * 128],
                                        rhs=QT[hsl, et,
                                               q0 + hf * 512:
                                               q0 + (hf + 1) * 512],
                                        start=True,
                                        stop=True,
                                    )
                                scs.append(sc)
                            for hh in range(2):
                                pt = PTP.tile([128, 1024], BF16, tag="pt")
                                if kt in DVE_KT:
                                    nc.vector.tensor_scalar(
                                        pt[:, 0:nq].bitcast(mybir.dt.int16),
                                        scs[hh][:, 0:nq],
                                        SCHRA_A, SCHRA_B,
                                        mybir.AluOpType.mult,
                                        mybir.AluOpType.add,
                                    )
                                else:
                                    nc.scalar.activation(
                                        pt[:, 0:nq], scs[hh][:, 0:nq], Exp,
                                        scale=SCALE,
                                    )
                                pts[hh, kt] = pt
                    # final block: slice-0 groups first so both finish with
                    # the exp stream; the trailing slice-1 pair borrows the
                    # (tail-idle) projection psum slots so all four groups
                    # accumulate concurrently instead of bursting after the
                    # last exp.
                    if last:
                        gorder = [(0, 0), (1, 0), (0, 1), (1, 1)]
                    else:
                        gorder = [(hh, sq) for hh in range(2)
                                  for sq in range(nhf)]
                    for hh, sq in gorder:
                        h = hp * 2 + hh
                        j = (hp * 2 + hh) * nhf + sq
                        borrow = last and hp == 1 and sq == 1
                        pool, tag = (PRJ, "prj") if borrow else (OPP, "ops")
                        ops = pool.tile([65, 512], F32, tag=tag)
                        for kt in range(KT):
                            nc.tensor.matmul(
                                ops[:, :],
                                lhsT=VA[:, kt, h * 65:(h + 1) * 65],
                                rhs=pts[hh, kt][:,
                                                sq * 512:(sq + 1) * 512],
                                start=(kt == 0),
                                stop=(kt == KT - 1),
                            )
                        # one copy moves the O rows and the denominator
                        # row together; the denominator then hops to its
                        # sums partition by DMA (DVE can't cross
                        # partitions).
                        ou = NRM.tile([65, 512], F32, tag="ou", bufs=9)
                        # tail copies ride the then-idle scalar engine
                        if last and hp == 1:
                            nc.scalar.copy(ou[:, :], ops[:, :])
                        else:
                            nc.vector.tensor_copy(ou[:, :], ops[:, :])
                        if last:
                            st = suA if sq == 0 else suB
                            srow = hp * 2 + hh
                            nc.sync.dma_start(
                                out=st[srow:srow + 1, :], in_=ou[64:65, :]
                            )
                        else:
                            nc.sync.dma_start(
                                out=sums[j:j + 1, :], in_=ou[64:65, :]
                            )
                        ous[j] = ou
                # keep the PE's HAM clock-gate warm across the reciprocal
                # latency at the very end: the accumulator slots are free
                # after the final ou copies, so a short burst of zero
                # matmuls (results never read) bridges PE activity until
                # the normalize/out-proj matmuls — which then run at full
                # clock instead of the re-throttled half rate.
                if last:
                    for _ in range(24):
                        wps = OPP.tile([65, 512], F32, tag="ops")
                        nc.tensor.matmul(
                            wps[0:64, 0:256],
                            lhsT=WRM[:, 0:64],
                            rhs=WRM[:, :],
                            start=True, stop=True,
                        )
                # normalize: mid-blocks batch all 8 reciprocal rows on the
                # DVE (they overlap the next block's attention); the final
                # block does them per query-slice on the by-then-idle scalar
                # engine (1/x = exp(-ln(x))), so slice 0's normalize and
                # out-projection overlap slice 1's trailing accumulation.
                if not last:
                    rcb = NRM.tile([8, 512], F32R, tag="rcb", bufs=2)
                    rcf = NRM.tile([8, 512], F32, tag="rcf", bufs=2)
                    with nc.allow_low_precision(
                        reason="softmax 1/denom rounded to fp32r for the "
                        "selector-matmul broadcast"
                    ):
                        nc.vector.reciprocal(rcf[0:nj, :], sums[0:nj, :])
                        nc.vector.tensor_copy(rcb[0:nj, :], rcf[0:nj, :])
                prio = tc.high_priority(offset=-500 if not last else 0)
                with prio:
                    for sq in range(nhf):
                        s0 = q0 + sq * 512
                        ssl = slice(s0, s0 + 512)
                        if last:
                            st = suA if sq == 0 else suB
                            lns = NRM.tile([8, 512], F32, tag="lns", bufs=2)
                            rcf = NRM.tile([8, 512], F32, tag="rcf", bufs=2)
                            rcb = NRM.tile([8, 512], F32R, tag="rcb", bufs=2)
                            nc.scalar.activation(lns[0:4, :], st[0:4, :], Ln)
                            nc.scalar.activation(rcf[0:4, :], lns[0:4, :],
                                                 Exp, scale=-1.0)
                            with nc.allow_low_precision(
                                reason="softmax 1/denom rounded to fp32r "
                                "for the selector-matmul broadcast"
                            ):
                                nc.vector.tensor_copy(rcb[0:4, :],
                                                      rcf[0:4, :])
                        for hp in range(2):
                            for hh in range(2):
                                hsl = slice(hh * 64, hh * 64 + 64)
                                c = hp * 2 + hh
                                j = c * nhf + sq
                                bc = PRJ.tile([64, 512], F32, tag="prj")
                                if last:
                                    nc.tensor.matmul(
                                        bc[:, :],
                                        lhsT=E8[0:4, c * 64:(c + 1) * 64],
                                        rhs=rcb[0:4, :],
                                        start=True, stop=True,
                                    )
                                else:
                                    nc.tensor.matmul(
                                        bc[:, :],
                                        lhsT=E8[0:8, j * 64:(j + 1) * 64],
                                        rhs=rcb[0:8, :],
                                        start=True, stop=True,
                                    )
                                nc.vector.tensor_mul(
                                    OT[hsl, hp, ssl], ous[j][0:64, :],
                                    bc[:, :]
                                )
                        for fc in range(4):
                            yp = PRJ.tile([128, 512], F32, tag="prj")
                            for et in range(2):
                                nc.tensor.matmul(
                                    yp[:, :],
                                    lhsT=WOT[:, et, fc * 128:(fc + 1) * 128],
                                    rhs=OT[:, et, ssl],
                                    start=(et == 0),
                                    stop=(et == 1),
                                )
                            ys = YO.tile([128, 512], F32, tag="ys")
                            if last and fc % 2 == 1:
                                nc.scalar.copy(ys[:, :], yp[:, :])
                            else:
                                nc.vector.tensor_copy(ys[:, :], yp[:, :])
                            nc.sync.dma_start(
                                out=yT[fc * 128:(fc + 1) * 128, ssl],
                                in_=ys[:, :],
                            )

    if sanitize:
        sanitize_waits(nc)
    return nc


def _perm_xt(x):
    # (S, D) -> x^T laid out [128, 4, S]: partition p, chunk dt = row
    # dt*128+p of x^T
    xt = x.T.astype(NP_BF16)                      # (512, S)
    return np.ascontiguousarray(
        xt.reshape(4, 128, S).transpose(1, 0, 2)
    )


def _e8():
    e = np.zeros((8, 512), dtype=np.float32)
    for j in range(8):
        e[j, j * 64:(j + 1) * 64] = 1.0
    return e


def make_in_maps(query, key, value, Wq, bq, Wk, bk, Wv, bv, Wo, bo):
    in_maps = []
    for c in range(NCORES):
        b, g = divmod(c, 2)
        eo = g * E
        esl = slice(eo, eo + E)
        in_maps.append({
            "xqT": _perm_xt(query[b]),
            "xkT": _perm_xt(key[b]),
            "xvT": _perm_xt(value[b]),
            "wqT": Wq[esl, :].T.astype(NP_BF16),
            "wkT": Wk[esl, :].T.astype(NP_BF16),
            "wvT": Wv[esl, :].T.astype(NP_BF16),
            "woT": Wo[:, esl].T.astype(NP_BF16),
            "bqs": np.ascontiguousarray(bq[esl], dtype=np.float32),
            "bks": np.ascontiguousarray(bk[esl], dtype=np.float32),
            "bvb": np.ascontiguousarray(
                np.broadcast_to(bv[esl], (128, E)), dtype=np.float32
            ),
            "e8d": _e8(),
        })
    return in_maps


def gather(results, bo):
    out = np.empty((B, S, D), dtype=np.float32)
    for b in range(B):
        yt = results[2 * b]["yT"] + results[2 * b + 1]["yT"]
        out[b] = yt.T + np.asarray(bo, dtype=np.float32)
    return out


_NC = None


def kernel(query, key, value, Wq, bq, Wk, bk, Wv, bv, Wo, bo, **run_kwargs):
    global _NC
    if _NC is None:
        _NC = build_nc()
    args = [np.asarray(a) for a in
            (query, key, value, Wq, bq, Wk, bk, Wv, bv, Wo, bo)]
    in_maps = make_in_maps(*args)
    res = run_bass_kernel_spmd(_NC, in_maps, list(range(NCORES)), **run_kwargs)
    out = gather(res.results, args[10])
    if run_kwargs:
        return out, res
    return out
